# revision 5
# baseline (speedup 1.0000x reference)
"""Trainium2 Bass kernel for nn_CoNe_35974646071945 (retrieval_knn).

Strategy: K-shard the 65536-entry queue across 8 NeuronCores. Host pre-casts
inputs (bf16 for the top-k sim path, fp8 e4m3 for the dc-target path) so the
device reads a quarter of the f32 bytes. Each core:
  simqT[j, b] = (queue_shard)^T @ norm_q^T      (bf16 matmul -> bf16 out)
  simkT[j, b] = (queue8_shard)^T @ k_feat8^T    (fp8 DoubleRow matmul)
  ET[j, b]    = exp(simkT / T_DC)               (fp8, SBUF-resident)
  P[b, cls+]  = ET^T @ [qlp8_shard^T * S | 1]   (fp8 DoubleRow matmuls,
                                                 fp32 PSUM accumulated over
                                                 the shard; ones column gives
                                                 the softmax partition Z)
Host: sums P partials over cores, exact top-200 / softmax / KL on tiny
arrays, returns the 3 losses.
"""
import sys
sys.path.insert(0, '/opt/trn_rl_repo')
sys.path.insert(0, '/root/.axon_site/_ro/trn_rl_repo')

import numpy as np
import ml_dtypes
from contextlib import ExitStack

from concourse import bass, tile, mybir
from concourse.bass_utils import run_bass_kernel_spmd
from concourse.vector_clock import ScopedClock, VectorClock

F32 = mybir.dt.float32
BF16 = mybir.dt.bfloat16
F8 = mybir.dt.float8e4
Alu = mybir.AluOpType
Act = mybir.ActivationFunctionType
DR = mybir.MatmulPerfMode.DoubleRow

NP_BF16 = ml_dtypes.bfloat16
NP_F8 = ml_dtypes.float8_e4m3

N_CORES = 8
B, D, K, C = 512, 256, 65536, 1000
KS = K // N_CORES            # 8192 queue columns per core
T_SUP, T_DC, LS = 0.07, 0.1, 0.1
EPS = 1e-8
NJT = KS // 128              # 64 j-tiles per core
NST = NJT // 2               # 32 super-tiles (256 j) for DoubleRow
C1 = 1008                    # 1000 classes + ones col (idx 1000) + pad to %16
CH = C1 // 2                 # 504-column matmul chunks (fits one PSUM bank)


class CompatTileContext(tile.TileContext):
    """This walrus build encodes at most ONE sync wait per instruction.
    Split Tile's multi-wait instructions and its tail drain."""

    def _commit_instruction(self, inst, lazy_reg_writes=True):
        si = inst.sync_info
        if (
            si is not None
            and si.on_wait
            and len(si.on_wait) > 1
            and inst.engine != mybir.EngineType.Unassigned
        ):
            import bass_rust
            waits = list(si.on_wait)
            for w in waits[:-1]:
                nop = mybir.InstNoOp(
                    name=f"I-{self.nc.next_id()}", ins=[], outs=[]
                )
                nop.engine = inst.engine
                nop.sync_info = bass_rust.SyncInfo(on_wait=[w], on_update=[])
                super()._commit_instruction(nop, lazy_reg_writes=False)
            si.on_wait = [waits[-1]]
            inst.sync_info = si
        super()._commit_instruction(inst, lazy_reg_writes=lazy_reg_writes)

    def _drain_and_barrier(self, tick_clock, wait_clock):
        gclock = tick_clock.global_clock
        n = len(gclock)
        for i in range(n):
            if gclock[i] == 0:
                continue
            vec = [0] * n
            vec[i] = gclock[i]
            nop_inst = self.nc.sync.nop(nofuse=True, hint=f"tail_wait_p{i}")
            wait_clock.add_sem_waits(
                nop_inst.ins, ScopedClock({None: VectorClock(vec)})
            )
        self.nc.sync.drain()
        self.nc.all_engine_barrier()
        assert self.sems is not None
        popped = self.nc._tile_sem_poison_stack.pop()
        assert popped is self._sem_poison
        self.nc.clear_and_free_semaphores(list(self.sems.allocated().values()))
        self.nc.all_engine_barrier()


_CACHED = {}


def _build():
    if 'nc' in _CACHED:
        return _CACHED['nc']
    nc = bass.Bass(num_devices=N_CORES)
    qT_in = nc.declare_dram_parameter("qT", [D, B], BF16, isOutput=False)
    kT8_in = nc.declare_dram_parameter("kT8", [D, B], F8, isOutput=False)
    qsh_in = nc.declare_dram_parameter("qsh", [D, KS], BF16, isOutput=False)
    qsh8_in = nc.declare_dram_parameter("qsh8", [D, KS], F8, isOutput=False)
    qlp8_in = nc.declare_dram_parameter(
        "qlp8", [NST * 128, 2, C1], F8, isOutput=False)
    simq_out = nc.declare_dram_parameter("simq", [KS, B], BF16, isOutput=True)
    p_out = nc.declare_dram_parameter("pout", [B, C1], F32, isOutput=True)

    with ExitStack() as ctx:
        tc = ctx.enter_context(CompatTileContext(nc))
        pool = ctx.enter_context(tc.tile_pool(name="main", bufs=1))
        qstg = ctx.enter_context(tc.tile_pool(name="qstg", bufs=NST))
        sq = ctx.enter_context(tc.tile_pool(name="sq", bufs=4))

        # moving operands: norm_q^T bf16 [d, 2, b]; k_feat^T fp8 [d, 2, b]
        qTb = pool.tile([128, 2, B], BF16, name="qTb")
        kT8 = pool.tile([128, 2, B], F8, name="kT8")
        for d in range(2):
            nc.sync.dma_start(qTb[:, d:d + 1, :], qT_in[d * 128:(d + 1) * 128, :])
            nc.sync.dma_start(kT8[:, d:d + 1, :], kT8_in[d * 128:(d + 1) * 128, :])

        # stationary queue shard: bf16 [d, 2, j] for simq, fp8 for simk
        qb = pool.tile([128, 2, KS], BF16, name="qb")
        q8 = pool.tile([128, 2, KS], F8, name="q8")
        for d in range(2):
            nc.sync.dma_start(qb[:, d:d + 1, :], qsh_in[d * 128:(d + 1) * 128, :])
            nc.sync.dma_start(q8[:, d:d + 1, :], qsh8_in[d * 128:(d + 1) * 128, :])

        # ET (exp(simk/T_DC)) fp8, [128, NJT, 512]
        et = pool.tile([128, NJT, B], F8, name="et")

        # phase 1: per j-tile simk (fp8 DoubleRow over d), exp, simq (bf16)
        with ExitStack() as ph1:
            psk = ph1.enter_context(
                tc.tile_pool(name="psk", bufs=2, space="PSUM"))
            psq = ph1.enter_context(
                tc.tile_pool(name="psq", bufs=2, space="PSUM"))
            for t in range(NJT):
                jl = t * 128
                pk = psk.tile([128, B], F32, name="pk", tag="pk")
                nc.tensor.matmul(
                    pk[:], q8[:, :, jl:jl + 128], kT8[:, :, :],
                    start=True, stop=True, perf_mode=DR)
                nc.scalar.activation(et[:, t:t + 1, :], pk[:],
                                     Act.Exp, scale=1.0 / T_DC)
                pq = psq.tile([128, B], F32, name="pq", tag="pq")
                for d in range(2):
                    nc.tensor.matmul(
                        pq[:], qb[:, d:d + 1, jl:jl + 128],
                        qTb[:, d:d + 1, :],
                        start=(d == 0), stop=(d == 1))
                sqt = sq.tile([128, B], BF16, name="sqt", tag="sqt")
                nc.vector.tensor_copy(sqt[:], pq[:])
                nc.sync.dma_start(simq_out[jl:jl + 128, :], sqt[:])

        # phase 2: P[b, cls+] = ET^T @ qlp8_aug, DoubleRow over 32 super-tiles
        with ExitStack() as ph2:
            ps2 = ph2.enter_context(
                tc.tile_pool(name="ps2", bufs=1, space="PSUM"))
            # [128, 1024] f32 = exactly 2 banks per tile, so each matmul
            # output chunk ([:, :512] / [:, 512:C1]) stays within one bank
            pacc = [ps2.tile([128, 1024], F32, name=f"pacc{bt}")
                    for bt in range(4)]
            for s in range(NST):
                ql = qstg.tile([128, 2, C1], F8, name="ql", tag="ql")
                nc.sync.dma_start(ql[:], qlp8_in[s * 128:(s + 1) * 128, :, :])
                for bt in range(4):
                    lhs = et[:, 2 * s:2 * s + 2, bt * 128:(bt + 1) * 128]
                    nc.tensor.matmul(pacc[bt][:, :512], lhs, ql[:, :, :512],
                                     start=(s == 0), stop=(s == NST - 1),
                                     perf_mode=DR)
                    nc.tensor.matmul(pacc[bt][:, 512:C1], lhs, ql[:, :, 512:],
                                     start=(s == 0), stop=(s == NST - 1),
                                     perf_mode=DR)
            for bt in range(4):
                pcp = sq.tile([128, C1], F32, name="pcp", tag="pcp")
                nc.vector.tensor_copy(pcp[:], pacc[bt][:, :C1])
                nc.sync.dma_start(p_out[bt * 128:(bt + 1) * 128, :], pcp[:])

    _CACHED['nc'] = nc
    return nc


def _prep_inputs(norm_q, k_feat, queue, qlp):
    """Host-side cast + layout. Returns (in_maps, S)."""
    mx = float(qlp.max())
    S = float(2.0 ** np.floor(np.log2(200.0 / max(mx, 1e-20))))
    qT = np.ascontiguousarray(norm_q.T).astype(NP_BF16)
    kT8 = np.ascontiguousarray(k_feat.T).astype(NP_F8)
    in_maps = []
    for c in range(N_CORES):
        sh = slice(c * KS, (c + 1) * KS)
        qs = np.ascontiguousarray(queue[:, sh])
        # qlp shard -> [KS, C1] fp8 with ones col + pad, then DoubleRow
        # interleave [NST*128, 2, C1]
        aug = np.zeros((KS, C1), np.float32)
        aug[:, :C] = qlp[:, sh].T * S
        aug[:, C] = 1.0
        aug8 = aug.astype(NP_F8)
        qlp8 = np.ascontiguousarray(
            aug8.reshape(NST, 2, 128, C1).transpose(0, 2, 1, 3)
        ).reshape(NST * 128, 2, C1)
        in_maps.append({
            "qT": qT, "kT8": kT8,
            "qsh": qs.astype(NP_BF16),
            "qsh8": qs.astype(NP_F8),
            "qlp8": qlp8,
        })
    return in_maps, S


def kernel(norm_q, q_logits, k_feat, logits_k, queue, queue_label_prob,
           queue_label, target, knn_k):
    norm_q = np.asarray(norm_q, np.float32)
    q_logits = np.asarray(q_logits, np.float32)
    k_feat = np.asarray(k_feat, np.float32)
    queue = np.asarray(queue, np.float32)
    qlp = np.asarray(queue_label_prob, np.float32)
    queue_label = np.asarray(queue_label)
    target = np.asarray(target)
    kk = int(knn_k)

    nc = _build()
    in_maps, S = _prep_inputs(norm_q, k_feat, queue, qlp)
    res = run_bass_kernel_spmd(nc, in_maps, list(range(N_CORES)))

    sim = np.concatenate(
        [res.results[c]["simq"].astype(np.float32).T
         for c in range(N_CORES)], axis=1)
    P = np.zeros((B, C1), np.float64)
    for c in range(N_CORES):
        P += res.results[c]["pout"].astype(np.float64)

    # ---- supcon (exact top-k on the device-computed sim) ----
    idx = np.argpartition(-sim, kk - 1, axis=1)[:, :kk]
    sim_knn = np.take_along_axis(sim, idx, axis=1)
    w = np.exp((sim_knn - sim_knn.max(axis=1, keepdims=True)) / T_SUP)
    w /= w.sum(axis=1, keepdims=True)
    pos = (target[:, None] == queue_label[idx])
    gt = (w * pos).sum(axis=1)
    m = gt > EPS
    supin_loss = np.where(m, -np.log(np.where(m, gt, 1.0)), 0.0).sum() / B

    # ---- fc loss ----
    x = q_logits.astype(np.float64)
    lse = np.log(np.exp(x - x.max(1, keepdims=True)).sum(1)) + x.max(1)
    log_q = x - lse[:, None]
    q_mask = (x.min(1) - lse) > np.log(EPS)
    onehot = np.full((B, C), LS / (C - 1))
    onehot[np.arange(B), target] = 1.0 - LS
    fc_loss = -((onehot * log_q).sum(1) * q_mask).sum() / B

    # ---- dc loss ----
    Z = P[:, C] * S
    dc_t = P[:, :C] / Z[:, None]
    dc_pos = dc_t > 0
    kl = np.where(dc_pos,
                  dc_t * (np.log(np.where(dc_pos, dc_t, 1.0)) - log_q), 0.0)
    dc_loss = (kl.sum(1) * q_mask).sum() / B

    return (np.float32(supin_loss), np.float32(fc_loss), np.float32(dc_loss))


# revision 8
# speedup vs baseline: 1.0943x; 1.0943x over previous
"""Trainium2 Bass kernel for nn_CoNe_35974646071945 (retrieval_knn).

Strategy: K-shard the 65536-entry queue across 8 NeuronCores. Host pre-casts
inputs (bf16 for the top-k sim path, fp8 e4m3 for the dc-target path) so the
device reads a quarter of the f32 bytes. Each core:
  simqT[j, b] = (queue_shard)^T @ norm_q^T      (bf16 matmul -> bf16 out)
  simkT[j, b] = (queue8_shard)^T @ k_feat8^T    (fp8 DoubleRow matmul)
  ET[j, b]    = exp(simkT / T_DC)               (fp8, SBUF-resident)
  P[b, cls+]  = ET^T @ [qlp8_shard^T * S | 1]   (fp8 DoubleRow matmuls,
                                                 fp32 PSUM accumulated over
                                                 the shard; ones column gives
                                                 the softmax partition Z)
Host: sums P partials over cores, exact top-200 / softmax / KL on tiny
arrays, returns the 3 losses.

Layout notes: queue loads are chunked along j so the first matmuls start
~2us in; simq writes are batched 8 j-tiles per DMA; qlp streams in 16
half-MB groups whose triggers are issued before phase 1 (prefetch). All
PSUM matmul outputs are single-bank (the 512/496 split of the 1008-wide
dc accumulation is load-bearing: a chunk straddling a 2KB PSUM bank
boundary corrupts the accumulation).
"""
import sys
sys.path.insert(0, '/opt/trn_rl_repo')
sys.path.insert(0, '/root/.axon_site/_ro/trn_rl_repo')

import numpy as np
import ml_dtypes
from contextlib import ExitStack

from concourse import bass, tile, mybir
from concourse.bass_utils import run_bass_kernel_spmd
from concourse.vector_clock import ScopedClock, VectorClock

F32 = mybir.dt.float32
BF16 = mybir.dt.bfloat16
F8 = mybir.dt.float8e4
Alu = mybir.AluOpType
Act = mybir.ActivationFunctionType
DR = mybir.MatmulPerfMode.DoubleRow

NP_BF16 = ml_dtypes.bfloat16
NP_F8 = ml_dtypes.float8_e4m3

N_CORES = 8
B, D, K, C = 512, 256, 65536, 1000
KS = K // N_CORES            # 8192 queue columns per core
T_SUP, T_DC, LS = 0.07, 0.1, 0.1
EPS = 1e-8
NJT = KS // 128              # 64 j-tiles per core
NST = NJT // 2               # 32 super-tiles (256 j) for DoubleRow
NG = NST // 2                # 16 qlp DMA groups (512 j each)
C1 = 1008                    # 1000 classes + ones col (idx 1000) + pad
NCH = 8                      # simq output chunks (8 j-tiles each)
NLC = 4                      # queue load chunks (2048 j-cols each)
LCW = KS // NLC


class CompatTileContext(tile.TileContext):
    """This walrus build encodes at most ONE sync wait per instruction.
    Split Tile's multi-wait instructions and its tail drain."""

    def _commit_instruction(self, inst, lazy_reg_writes=True):
        si = inst.sync_info
        if (
            si is not None
            and si.on_wait
            and len(si.on_wait) > 1
            and inst.engine != mybir.EngineType.Unassigned
        ):
            import bass_rust
            waits = list(si.on_wait)
            for w in waits[:-1]:
                nop = mybir.InstNoOp(
                    name=f"I-{self.nc.next_id()}", ins=[], outs=[]
                )
                nop.engine = inst.engine
                nop.sync_info = bass_rust.SyncInfo(on_wait=[w], on_update=[])
                super()._commit_instruction(nop, lazy_reg_writes=False)
            si.on_wait = [waits[-1]]
            inst.sync_info = si
        super()._commit_instruction(inst, lazy_reg_writes=lazy_reg_writes)

    def _drain_and_barrier(self, tick_clock, wait_clock):
        gclock = tick_clock.global_clock
        n = len(gclock)
        for i in range(n):
            if gclock[i] == 0:
                continue
            vec = [0] * n
            vec[i] = gclock[i]
            nop_inst = self.nc.sync.nop(nofuse=True, hint=f"tail_wait_p{i}")
            wait_clock.add_sem_waits(
                nop_inst.ins, ScopedClock({None: VectorClock(vec)})
            )
        self.nc.sync.drain()
        self.nc.all_engine_barrier()
        assert self.sems is not None
        popped = self.nc._tile_sem_poison_stack.pop()
        assert popped is self._sem_poison
        self.nc.clear_and_free_semaphores(list(self.sems.allocated().values()))
        self.nc.all_engine_barrier()


_CACHED = {}


def _build():
    if 'nc' in _CACHED:
        return _CACHED['nc']
    nc = bass.Bass(num_devices=N_CORES)
    qT_in = nc.declare_dram_parameter("qT", [D, B], BF16, isOutput=False)
    kT8_in = nc.declare_dram_parameter("kT8", [D, B], F8, isOutput=False)
    qsh_in = nc.declare_dram_parameter("qsh", [D, KS], BF16, isOutput=False)
    qsh8_in = nc.declare_dram_parameter("qsh8", [D, KS], F8, isOutput=False)
    qlp8_in = nc.declare_dram_parameter(
        "qlp8", [NG * 128, 4, C1], F8, isOutput=False)
    # simq[c*128+p, sub, b] = sim[j = c*1024 + sub*128 + p, b]
    simq_out = nc.declare_dram_parameter(
        "simq", [NCH * 128, NJT // NCH, B], BF16, isOutput=True)
    p_out = nc.declare_dram_parameter("pout", [B, C1], F32, isOutput=True)

    with ExitStack() as ctx:
        tc = ctx.enter_context(CompatTileContext(nc))
        pool = ctx.enter_context(tc.tile_pool(name="main", bufs=1))
        qstg = ctx.enter_context(tc.tile_pool(name="qstg", bufs=1))
        sq = ctx.enter_context(tc.tile_pool(name="sq", bufs=2))
        pc = ctx.enter_context(tc.tile_pool(name="pc", bufs=4))

        # moving operands first: norm_q^T bf16 / k_feat^T fp8, [d, 2, b]
        qTb = pool.tile([128, 2, B], BF16, name="qTb")
        kT8 = pool.tile([128, 2, B], F8, name="kT8")
        for d in range(2):
            nc.sync.dma_start(kT8[:, d:d + 1, :], kT8_in[d * 128:(d + 1) * 128, :])
            nc.sync.dma_start(qTb[:, d:d + 1, :], qT_in[d * 128:(d + 1) * 128, :])

        # stationary queue shard, chunked along j so matmuls start early
        qb = pool.tile([128, 2, KS], BF16, name="qb")
        q8 = pool.tile([128, 2, KS], F8, name="q8")
        for lc in range(NLC):
            jsl = slice(lc * LCW, (lc + 1) * LCW)
            for d in range(2):
                nc.sync.dma_start(q8[:, d:d + 1, jsl],
                                  qsh8_in[d * 128:(d + 1) * 128, jsl])
            for d in range(2):
                nc.sync.dma_start(qb[:, d:d + 1, jsl],
                                  qsh_in[d * 128:(d + 1) * 128, jsl])

        # qlp prefetch: no deps, issued up front so phase 2 never starves
        qls = []
        for g in range(NG):
            ql = qstg.tile([128, 4, C1], F8, name=f"ql{g}")
            nc.sync.dma_start(ql[:], qlp8_in[g * 128:(g + 1) * 128, :, :])
            qls.append(ql)

        # ET (exp(simk/T_DC)) fp8, [128, NJT, 512]
        et = pool.tile([128, NJT, B], F8, name="et")

        # phase 1: per j-tile simk (fp8 DoubleRow over d), exp, simq (bf16)
        with ExitStack() as ph1:
            psk = ph1.enter_context(
                tc.tile_pool(name="psk", bufs=2, space="PSUM"))
            psq = ph1.enter_context(
                tc.tile_pool(name="psq", bufs=2, space="PSUM"))
            for ch in range(NCH):
                sqt = sq.tile([128, NJT // NCH, B], BF16, name="sqt", tag="sqt")
                for sub in range(NJT // NCH):
                    t = ch * (NJT // NCH) + sub
                    jl = t * 128
                    pk = psk.tile([128, B], F32, name="pk", tag="pk")
                    nc.tensor.matmul(
                        pk[:], q8[:, :, jl:jl + 128], kT8[:, :, :],
                        start=True, stop=True, perf_mode=DR)
                    nc.scalar.activation(et[:, t:t + 1, :], pk[:],
                                         Act.Exp, scale=1.0 / T_DC)
                    pq = psq.tile([128, B], F32, name="pq", tag="pq")
                    for d in range(2):
                        nc.tensor.matmul(
                            pq[:], qb[:, d:d + 1, jl:jl + 128],
                            qTb[:, d:d + 1, :],
                            start=(d == 0), stop=(d == 1))
                    nc.vector.tensor_copy(sqt[:, sub:sub + 1, :], pq[:])
                nc.sync.dma_start(
                    simq_out[ch * 128:(ch + 1) * 128, :, :], sqt[:])

        # phase 2: P[b, cls+] = ET^T @ qlp8_aug, DoubleRow over 32 super-tiles
        with ExitStack() as ph2:
            ps2 = ph2.enter_context(
                tc.tile_pool(name="ps2", bufs=1, space="PSUM"))
            # [128, 1024] f32 = exactly 2 banks per tile: each matmul chunk
            # ([:, :512] / [:, 512:C1]) stays within one bank
            pacc = [ps2.tile([128, 1024], F32, name=f"pacc{bt}")
                    for bt in range(4)]
            for g in range(NG):
                ql = qls[g]
                for sl in range(2):
                    s = 2 * g + sl
                    st = (s == 0)
                    sp = (s == NST - 1)
                    for bt in range(4):
                        lhs = et[:, 2 * s:2 * s + 2, bt * 128:(bt + 1) * 128]
                        nc.tensor.matmul(
                            pacc[bt][:, :512], lhs,
                            ql[:, 2 * sl:2 * sl + 2, :512],
                            start=st, stop=sp, perf_mode=DR)
                        nc.tensor.matmul(
                            pacc[bt][:, 512:C1], lhs,
                            ql[:, 2 * sl:2 * sl + 2, 512:],
                            start=st, stop=sp, perf_mode=DR)
            for bt in range(4):
                pcp = pc.tile([128, C1], F32, name="pcp", tag="pcp")
                if bt % 2 == 0:
                    nc.vector.tensor_copy(pcp[:], pacc[bt][:, :C1])
                else:
                    nc.scalar.activation(pcp[:], pacc[bt][:, :C1],
                                         Act.Copy, scale=1.0)
                nc.sync.dma_start(p_out[bt * 128:(bt + 1) * 128, :], pcp[:])

    _CACHED['nc'] = nc
    return nc


def _prep_inputs(norm_q, k_feat, queue, qlp):
    """Host-side cast + layout. Returns (in_maps, S)."""
    mx = float(qlp.max())
    S = float(2.0 ** np.floor(np.log2(200.0 / max(mx, 1e-20))))
    qT = np.ascontiguousarray(norm_q.T).astype(NP_BF16)
    kT8 = np.ascontiguousarray(k_feat.T).astype(NP_F8)
    in_maps = []
    for c in range(N_CORES):
        sh = slice(c * KS, (c + 1) * KS)
        qs = np.ascontiguousarray(queue[:, sh])
        # qlp shard -> [KS, C1] fp8 with ones col + pad, then grouped
        # DoubleRow interleave [NG*128, 4, C1]
        aug = np.zeros((KS, C1), np.float32)
        aug[:, :C] = qlp[:, sh].T * S
        aug[:, C] = 1.0
        aug8 = aug.astype(NP_F8)
        qlp8 = np.ascontiguousarray(
            aug8.reshape(NG, 2, 2, 128, C1).transpose(0, 3, 1, 2, 4)
        ).reshape(NG * 128, 4, C1)
        in_maps.append({
            "qT": qT, "kT8": kT8,
            "qsh": qs.astype(NP_BF16),
            "qsh8": qs.astype(NP_F8),
            "qlp8": qlp8,
        })
    return in_maps, S


def kernel(norm_q, q_logits, k_feat, logits_k, queue, queue_label_prob,
           queue_label, target, knn_k):
    norm_q = np.asarray(norm_q, np.float32)
    q_logits = np.asarray(q_logits, np.float32)
    k_feat = np.asarray(k_feat, np.float32)
    queue = np.asarray(queue, np.float32)
    qlp = np.asarray(queue_label_prob, np.float32)
    queue_label = np.asarray(queue_label)
    target = np.asarray(target)
    kk = int(knn_k)

    nc = _build()
    in_maps, S = _prep_inputs(norm_q, k_feat, queue, qlp)
    res = run_bass_kernel_spmd(nc, in_maps, list(range(N_CORES)))

    # simq[c, p, sub, b] -> sim rows j = c*1024 + sub*128 + p
    blocks = []
    for c in range(N_CORES):
        A = res.results[c]["simq"].astype(np.float32)
        A = A.reshape(NCH, 128, NJT // NCH, B).transpose(0, 2, 1, 3)
        blocks.append(A.reshape(KS, B).T)
    sim = np.concatenate(blocks, axis=1)
    P = np.zeros((B, C1), np.float64)
    for c in range(N_CORES):
        P += res.results[c]["pout"].astype(np.float64)

    # ---- supcon (exact top-k on the device-computed sim) ----
    idx = np.argpartition(-sim, kk - 1, axis=1)[:, :kk]
    sim_knn = np.take_along_axis(sim, idx, axis=1)
    w = np.exp((sim_knn - sim_knn.max(axis=1, keepdims=True)) / T_SUP)
    w /= w.sum(axis=1, keepdims=True)
    pos = (target[:, None] == queue_label[idx])
    gt = (w * pos).sum(axis=1)
    m = gt > EPS
    supin_loss = np.where(m, -np.log(np.where(m, gt, 1.0)), 0.0).sum() / B

    # ---- fc loss ----
    x = q_logits.astype(np.float64)
    lse = np.log(np.exp(x - x.max(1, keepdims=True)).sum(1)) + x.max(1)
    log_q = x - lse[:, None]
    q_mask = (x.min(1) - lse) > np.log(EPS)
    onehot = np.full((B, C), LS / (C - 1))
    onehot[np.arange(B), target] = 1.0 - LS
    fc_loss = -((onehot * log_q).sum(1) * q_mask).sum() / B

    # ---- dc loss ----
    Z = P[:, C] * S
    dc_t = P[:, :C] / Z[:, None]
    dc_pos = dc_t > 0
    kl = np.where(dc_pos,
                  dc_t * (np.log(np.where(dc_pos, dc_t, 1.0)) - log_q), 0.0)
    dc_loss = (kl.sum(1) * q_mask).sum() / B

    return (np.float32(supin_loss), np.float32(fc_loss), np.float32(dc_loss))


# revision 10
# speedup vs baseline: 1.2256x; 1.1200x over previous
"""Trainium2 Bass kernel for nn_CoNe_35974646071945 (retrieval_knn).

Strategy: K-shard the 65536-entry queue across 8 NeuronCores. Host pre-casts
inputs (bf16 for the top-k sim path, fp8 e4m3 for the dc-target path) so the
device reads a quarter of the f32 bytes. Each core:
  simqT[j, b] = (queue_shard)^T @ norm_q^T      (bf16 matmul -> bf16 out)
  simkT[j, b] = (queue8_shard)^T @ k_feat8^T    (fp8 DoubleRow matmul)
  ET[j, b]    = exp(simkT / T_DC)               (fp8, SBUF-resident)
  P[b, cls+]  = ET^T @ [qlp8_shard^T * S | 1]   (fp8 DoubleRow matmuls,
                                                 fp32 PSUM accumulated over
                                                 the shard; ones column gives
                                                 the softmax partition Z)
Host: sums P partials over cores, exact top-200 / softmax / KL on tiny
arrays, returns the 3 losses.

Layout notes: queue loads are chunked along j so the first matmuls start
~2us in; simq writes are batched 8 j-tiles per DMA; qlp streams in 16
half-MB groups whose triggers are issued before phase 1 (prefetch). All
PSUM matmul outputs are single-bank (the 512/496 split of the 1008-wide
dc accumulation is load-bearing: a chunk straddling a 2KB PSUM bank
boundary corrupts the accumulation).
"""
import sys
sys.path.insert(0, '/opt/trn_rl_repo')
sys.path.insert(0, '/root/.axon_site/_ro/trn_rl_repo')

import numpy as np
import ml_dtypes
from contextlib import ExitStack

from concourse import bass, tile, mybir
from concourse.bass_utils import run_bass_kernel_spmd
from concourse.vector_clock import ScopedClock, VectorClock

F32 = mybir.dt.float32
BF16 = mybir.dt.bfloat16
F8 = mybir.dt.float8e4
Alu = mybir.AluOpType
Act = mybir.ActivationFunctionType
DR = mybir.MatmulPerfMode.DoubleRow

NP_BF16 = ml_dtypes.bfloat16
NP_F8 = ml_dtypes.float8_e4m3

N_CORES = 8
B, D, K, C = 512, 256, 65536, 1000
KS = K // N_CORES            # 8192 queue columns per core
T_SUP, T_DC, LS = 0.07, 0.1, 0.1
EPS = 1e-8
NJT = KS // 128              # 64 j-tiles per core
NST = NJT // 2               # 32 super-tiles (256 j) for DoubleRow
NG = NST // 2                # 16 qlp DMA groups (512 j each)
C1 = 1008                    # 1000 classes + ones col (idx 1000) + pad
NCH = 8                      # simq output chunks (8 j-tiles each)
NLC = 4                      # queue load chunks (2048 j-cols each)
LCW = KS // NLC


class CompatTileContext(tile.TileContext):
    """This walrus build encodes at most ONE sync wait per instruction.
    Split Tile's multi-wait instructions and its tail drain."""

    def _commit_instruction(self, inst, lazy_reg_writes=True):
        si = inst.sync_info
        if (
            si is not None
            and si.on_wait
            and len(si.on_wait) > 1
            and inst.engine != mybir.EngineType.Unassigned
        ):
            import bass_rust
            waits = list(si.on_wait)
            for w in waits[:-1]:
                nop = mybir.InstNoOp(
                    name=f"I-{self.nc.next_id()}", ins=[], outs=[]
                )
                nop.engine = inst.engine
                nop.sync_info = bass_rust.SyncInfo(on_wait=[w], on_update=[])
                super()._commit_instruction(nop, lazy_reg_writes=False)
            si.on_wait = [waits[-1]]
            inst.sync_info = si
        super()._commit_instruction(inst, lazy_reg_writes=lazy_reg_writes)

    def _drain_and_barrier(self, tick_clock, wait_clock):
        gclock = tick_clock.global_clock
        n = len(gclock)
        for i in range(n):
            if gclock[i] == 0:
                continue
            vec = [0] * n
            vec[i] = gclock[i]
            nop_inst = self.nc.sync.nop(nofuse=True, hint=f"tail_wait_p{i}")
            wait_clock.add_sem_waits(
                nop_inst.ins, ScopedClock({None: VectorClock(vec)})
            )
        self.nc.sync.drain()
        self.nc.all_engine_barrier()
        assert self.sems is not None
        popped = self.nc._tile_sem_poison_stack.pop()
        assert popped is self._sem_poison
        self.nc.clear_and_free_semaphores(list(self.sems.allocated().values()))
        self.nc.all_engine_barrier()


_CACHED = {}


def _build():
    if 'nc' in _CACHED:
        return _CACHED['nc']
    nc = bass.Bass(num_devices=N_CORES)
    qT_in = nc.declare_dram_parameter("qT", [D, B], BF16, isOutput=False)
    kT8_in = nc.declare_dram_parameter("kT8", [D, B], F8, isOutput=False)
    qsh_in = nc.declare_dram_parameter("qsh", [D, KS], BF16, isOutput=False)
    qsh8_in = nc.declare_dram_parameter("qsh8", [D, KS], F8, isOutput=False)
    qlp8_in = nc.declare_dram_parameter(
        "qlp8", [NG * 128, 4, C1], F8, isOutput=False)
    # simq[c*128+p, sub, b] = sim[j = c*1024 + sub*128 + p, b]
    simq_out = nc.declare_dram_parameter(
        "simq", [NCH * 128, NJT // NCH, B], BF16, isOutput=True)
    p_out = nc.declare_dram_parameter("pout", [B, C1], F32, isOutput=True)

    with ExitStack() as ctx:
        tc = ctx.enter_context(CompatTileContext(nc))
        pool = ctx.enter_context(tc.tile_pool(name="main", bufs=1))
        qstg = ctx.enter_context(tc.tile_pool(name="qstg", bufs=1))
        sq = ctx.enter_context(tc.tile_pool(name="sq", bufs=2))
        pc = ctx.enter_context(tc.tile_pool(name="pc", bufs=4))

        # moving operands first: norm_q^T bf16 / k_feat^T fp8, [d, 2, b]
        qTb = pool.tile([128, 2, B], BF16, name="qTb")
        kT8 = pool.tile([128, 2, B], F8, name="kT8")
        for d in range(2):
            nc.sync.dma_start(kT8[:, d:d + 1, :], kT8_in[d * 128:(d + 1) * 128, :])
            nc.sync.dma_start(qTb[:, d:d + 1, :], qT_in[d * 128:(d + 1) * 128, :])

        # stationary queue shard, chunked along j so matmuls start early
        qb = pool.tile([128, 2, KS], BF16, name="qb")
        q8 = pool.tile([128, 2, KS], F8, name="q8")
        for lc in range(NLC):
            jsl = slice(lc * LCW, (lc + 1) * LCW)
            for d in range(2):
                nc.sync.dma_start(q8[:, d:d + 1, jsl],
                                  qsh8_in[d * 128:(d + 1) * 128, jsl])
            for d in range(2):
                nc.sync.dma_start(qb[:, d:d + 1, jsl],
                                  qsh_in[d * 128:(d + 1) * 128, jsl])

        # qlp tiles: DMAs are issued rate-matched inside the phase-1 loop
        # (two groups per simq chunk) so they neither starve phase-1's
        # write-backs nor leave phase 2 waiting
        qls = [qstg.tile([128, 4, C1], F8, name=f"ql{g}") for g in range(NG)]

        # ET (exp(simk/T_DC)) fp8, [128, NJT, 512]
        et = pool.tile([128, NJT, B], F8, name="et")

        # phase 1: per j-tile simk (fp8 DoubleRow over d), exp, simq (bf16)
        with ExitStack() as ph1:
            psk = ph1.enter_context(
                tc.tile_pool(name="psk", bufs=2, space="PSUM"))
            psq = ph1.enter_context(
                tc.tile_pool(name="psq", bufs=2, space="PSUM"))
            for ch in range(NCH):
                sqt = sq.tile([128, NJT // NCH, B], BF16, name="sqt", tag="sqt")
                for sub in range(NJT // NCH):
                    t = ch * (NJT // NCH) + sub
                    jl = t * 128
                    pk = psk.tile([128, B], F32, name="pk", tag="pk")
                    nc.tensor.matmul(
                        pk[:], q8[:, :, jl:jl + 128], kT8[:, :, :],
                        start=True, stop=True, perf_mode=DR)
                    nc.scalar.activation(et[:, t:t + 1, :], pk[:],
                                         Act.Exp, scale=1.0 / T_DC)
                    pq = psq.tile([128, B], F32, name="pq", tag="pq")
                    for d in range(2):
                        nc.tensor.matmul(
                            pq[:], qb[:, d:d + 1, jl:jl + 128],
                            qTb[:, d:d + 1, :],
                            start=(d == 0), stop=(d == 1))
                    nc.vector.tensor_copy(sqt[:, sub:sub + 1, :], pq[:])
                nc.sync.dma_start(
                    simq_out[ch * 128:(ch + 1) * 128, :, :], sqt[:])
                for g in (2 * ch, 2 * ch + 1):
                    nc.sync.dma_start(
                        qls[g][:], qlp8_in[g * 128:(g + 1) * 128, :, :])

        # phase 2: P[b, cls+] = ET^T @ qlp8_aug, DoubleRow over 32 super-tiles
        with ExitStack() as ph2:
            ps2 = ph2.enter_context(
                tc.tile_pool(name="ps2", bufs=1, space="PSUM"))
            # [128, 1024] f32 = exactly 2 banks per tile: each matmul chunk
            # ([:, :512] / [:, 512:C1]) stays within one bank
            pacc = [ps2.tile([128, 1024], F32, name=f"pacc{bt}")
                    for bt in range(4)]
            for g in range(NG):
                ql = qls[g]
                for sl in range(2):
                    s = 2 * g + sl
                    st = (s == 0)
                    sp = (s == NST - 1)
                    for bt in range(4):
                        lhs = et[:, 2 * s:2 * s + 2, bt * 128:(bt + 1) * 128]
                        nc.tensor.matmul(
                            pacc[bt][:, :512], lhs,
                            ql[:, 2 * sl:2 * sl + 2, :512],
                            start=st, stop=sp, perf_mode=DR)
                        nc.tensor.matmul(
                            pacc[bt][:, 512:C1], lhs,
                            ql[:, 2 * sl:2 * sl + 2, 512:],
                            start=st, stop=sp, perf_mode=DR)
            for bt in range(4):
                pcp = pc.tile([128, C1], F32, name="pcp", tag="pcp")
                if bt % 2 == 0:
                    nc.vector.tensor_copy(pcp[:], pacc[bt][:, :C1])
                else:
                    nc.scalar.activation(pcp[:], pacc[bt][:, :C1],
                                         Act.Copy, scale=1.0)
                nc.sync.dma_start(p_out[bt * 128:(bt + 1) * 128, :], pcp[:])

    _CACHED['nc'] = nc
    return nc


def _prep_inputs(norm_q, k_feat, queue, qlp):
    """Host-side cast + layout. Returns (in_maps, S)."""
    mx = float(qlp.max())
    S = float(2.0 ** np.floor(np.log2(200.0 / max(mx, 1e-20))))
    qT = np.ascontiguousarray(norm_q.T).astype(NP_BF16)
    kT8 = np.ascontiguousarray(k_feat.T).astype(NP_F8)
    in_maps = []
    for c in range(N_CORES):
        sh = slice(c * KS, (c + 1) * KS)
        qs = np.ascontiguousarray(queue[:, sh])
        # qlp shard -> [KS, C1] fp8 with ones col + pad, then grouped
        # DoubleRow interleave [NG*128, 4, C1]
        aug = np.zeros((KS, C1), np.float32)
        aug[:, :C] = qlp[:, sh].T * S
        aug[:, C] = 1.0
        aug8 = aug.astype(NP_F8)
        qlp8 = np.ascontiguousarray(
            aug8.reshape(NG, 2, 2, 128, C1).transpose(0, 3, 1, 2, 4)
        ).reshape(NG * 128, 4, C1)
        in_maps.append({
            "qT": qT, "kT8": kT8,
            "qsh": qs.astype(NP_BF16),
            "qsh8": qs.astype(NP_F8),
            "qlp8": qlp8,
        })
    return in_maps, S


def kernel(norm_q, q_logits, k_feat, logits_k, queue, queue_label_prob,
           queue_label, target, knn_k):
    norm_q = np.asarray(norm_q, np.float32)
    q_logits = np.asarray(q_logits, np.float32)
    k_feat = np.asarray(k_feat, np.float32)
    queue = np.asarray(queue, np.float32)
    qlp = np.asarray(queue_label_prob, np.float32)
    queue_label = np.asarray(queue_label)
    target = np.asarray(target)
    kk = int(knn_k)

    nc = _build()
    in_maps, S = _prep_inputs(norm_q, k_feat, queue, qlp)
    res = run_bass_kernel_spmd(nc, in_maps, list(range(N_CORES)))

    # simq[c, p, sub, b] -> sim rows j = c*1024 + sub*128 + p
    blocks = []
    for c in range(N_CORES):
        A = res.results[c]["simq"].astype(np.float32)
        A = A.reshape(NCH, 128, NJT // NCH, B).transpose(0, 2, 1, 3)
        blocks.append(A.reshape(KS, B).T)
    sim = np.concatenate(blocks, axis=1)
    P = np.zeros((B, C1), np.float64)
    for c in range(N_CORES):
        P += res.results[c]["pout"].astype(np.float64)

    # ---- supcon (exact top-k on the device-computed sim) ----
    idx = np.argpartition(-sim, kk - 1, axis=1)[:, :kk]
    sim_knn = np.take_along_axis(sim, idx, axis=1)
    w = np.exp((sim_knn - sim_knn.max(axis=1, keepdims=True)) / T_SUP)
    w /= w.sum(axis=1, keepdims=True)
    pos = (target[:, None] == queue_label[idx])
    gt = (w * pos).sum(axis=1)
    m = gt > EPS
    supin_loss = np.where(m, -np.log(np.where(m, gt, 1.0)), 0.0).sum() / B

    # ---- fc loss ----
    x = q_logits.astype(np.float64)
    lse = np.log(np.exp(x - x.max(1, keepdims=True)).sum(1)) + x.max(1)
    log_q = x - lse[:, None]
    q_mask = (x.min(1) - lse) > np.log(EPS)
    onehot = np.full((B, C), LS / (C - 1))
    onehot[np.arange(B), target] = 1.0 - LS
    fc_loss = -((onehot * log_q).sum(1) * q_mask).sum() / B

    # ---- dc loss ----
    Z = P[:, C] * S
    dc_t = P[:, :C] / Z[:, None]
    dc_pos = dc_t > 0
    kl = np.where(dc_pos,
                  dc_t * (np.log(np.where(dc_pos, dc_t, 1.0)) - log_q), 0.0)
    dc_loss = (kl.sum(1) * q_mask).sum() / B

    return (np.float32(supin_loss), np.float32(fc_loss), np.float32(dc_loss))


# revision 13
# speedup vs baseline: 1.3436x; 1.0963x over previous
"""Trainium2 Bass kernel for nn_CoNe_35974646071945 (retrieval_knn).

Strategy: K-shard the 65536-entry queue across 8 NeuronCores. Host pre-casts
inputs (bf16 for the top-k sim path, fp8 e4m3 for the dc-target path) so the
device reads a quarter of the f32 bytes. Each core:
  simqT[j, b] = (queue_shard)^T @ norm_q^T      (bf16 matmul -> bf16 out)
  simkT[j, b] = (queue8_shard)^T @ k_feat8^T    (fp8 DoubleRow matmul)
  ET[j, b]    = exp(simkT / T_DC)               (fp8, SBUF-resident)
  P[b, cls+]  = ET^T @ [qlp8_shard^T * S | 1]   (fp8 DoubleRow matmuls,
                                                 fp32 PSUM accumulated over
                                                 the shard; ones column gives
                                                 the softmax partition Z)
Host: sums P partials over cores, exact top-200 / softmax / KL on tiny
arrays, returns the 3 losses.

Layout notes: queue loads are chunked along j so the first matmuls start
~2us in; simq writes are batched 8 j-tiles per DMA; qlp streams in 16
half-MB groups whose triggers are issued before phase 1 (prefetch). All
PSUM matmul outputs are single-bank (the 512/496 split of the 1008-wide
dc accumulation is load-bearing: a chunk straddling a 2KB PSUM bank
boundary corrupts the accumulation).
"""
import sys
sys.path.insert(0, '/opt/trn_rl_repo')
sys.path.insert(0, '/root/.axon_site/_ro/trn_rl_repo')

import numpy as np
import ml_dtypes
from contextlib import ExitStack

from concourse import bass, tile, mybir
from concourse.bass_utils import run_bass_kernel_spmd
from concourse.vector_clock import ScopedClock, VectorClock

F32 = mybir.dt.float32
BF16 = mybir.dt.bfloat16
F8 = mybir.dt.float8e4
Alu = mybir.AluOpType
Act = mybir.ActivationFunctionType
DR = mybir.MatmulPerfMode.DoubleRow

NP_BF16 = ml_dtypes.bfloat16
NP_F8 = ml_dtypes.float8_e4m3

N_CORES = 8
B, D, K, C = 512, 256, 65536, 1000
KS = K // N_CORES            # 8192 queue columns per core
T_SUP, T_DC, LS = 0.07, 0.1, 0.1
EPS = 1e-8
NJT = KS // 128              # 64 j-tiles per core
NST = NJT // 2               # 32 super-tiles (256 j) for DoubleRow
NG = NST // 2                # 16 qlp DMA groups (512 j each)
C1 = 1008                    # 1000 classes + ones col (idx 1000) + pad
NCH = 8                      # simq output chunks (8 j-tiles each)
NLC = 4                      # queue load chunks (2048 j-cols each)
LCW = KS // NLC


class CompatTileContext(tile.TileContext):
    """This walrus build encodes at most ONE sync wait per instruction.
    Split Tile's multi-wait instructions and its tail drain."""

    def _commit_instruction(self, inst, lazy_reg_writes=True):
        si = inst.sync_info
        if (
            si is not None
            and si.on_wait
            and len(si.on_wait) > 1
            and inst.engine != mybir.EngineType.Unassigned
        ):
            import bass_rust
            waits = list(si.on_wait)
            for w in waits[:-1]:
                nop = mybir.InstNoOp(
                    name=f"I-{self.nc.next_id()}", ins=[], outs=[]
                )
                nop.engine = inst.engine
                nop.sync_info = bass_rust.SyncInfo(on_wait=[w], on_update=[])
                super()._commit_instruction(nop, lazy_reg_writes=False)
            si.on_wait = [waits[-1]]
            inst.sync_info = si
        super()._commit_instruction(inst, lazy_reg_writes=lazy_reg_writes)

    def _drain_and_barrier(self, tick_clock, wait_clock):
        gclock = tick_clock.global_clock
        n = len(gclock)
        for i in range(n):
            if gclock[i] == 0:
                continue
            vec = [0] * n
            vec[i] = gclock[i]
            nop_inst = self.nc.sync.nop(nofuse=True, hint=f"tail_wait_p{i}")
            wait_clock.add_sem_waits(
                nop_inst.ins, ScopedClock({None: VectorClock(vec)})
            )
        self.nc.sync.drain()
        self.nc.all_engine_barrier()
        assert self.sems is not None
        popped = self.nc._tile_sem_poison_stack.pop()
        assert popped is self._sem_poison
        self.nc.clear_and_free_semaphores(list(self.sems.allocated().values()))
        self.nc.all_engine_barrier()


_CACHED = {}


def _build():
    if 'nc' in _CACHED:
        return _CACHED['nc']
    nc = bass.Bass(num_devices=N_CORES)
    qT_in = nc.declare_dram_parameter("qT", [D, B], BF16, isOutput=False)
    kT8_in = nc.declare_dram_parameter("kT8", [D, B], F8, isOutput=False)
    qsh_in = nc.declare_dram_parameter("qsh", [D, KS], BF16, isOutput=False)
    qsh8_in = nc.declare_dram_parameter("qsh8", [D, KS], F8, isOutput=False)
    qlp8_in = nc.declare_dram_parameter(
        "qlp8", [NG * 128, 4, C1], F8, isOutput=False)
    # simq[c*128+p, sub, b] = sim[j = c*1024 + sub*128 + p, b]
    simq_out = nc.declare_dram_parameter(
        "simq", [NCH * 128, NJT // NCH, B], BF16, isOutput=True)
    p_out = nc.declare_dram_parameter("pout", [B, C1], F32, isOutput=True)

    with ExitStack() as ctx:
        tc = ctx.enter_context(CompatTileContext(nc))
        pool = ctx.enter_context(tc.tile_pool(name="main", bufs=1))
        qstg = ctx.enter_context(tc.tile_pool(name="qstg", bufs=1))
        sq = ctx.enter_context(tc.tile_pool(name="sq", bufs=3))
        pc = ctx.enter_context(tc.tile_pool(name="pc", bufs=4))

        # moving operands first: norm_q^T bf16 / k_feat^T fp8, [d, 2, b]
        qTb = pool.tile([128, 2, B], BF16, name="qTb")
        kT8 = pool.tile([128, 2, B], F8, name="kT8")
        for d in range(2):
            nc.sync.dma_start(kT8[:, d:d + 1, :], kT8_in[d * 128:(d + 1) * 128, :])
            nc.sync.dma_start(qTb[:, d:d + 1, :], qT_in[d * 128:(d + 1) * 128, :])

        # stationary queue shard, chunked along j so matmuls start early
        qb = pool.tile([128, 2, KS], BF16, name="qb")
        q8 = pool.tile([128, 2, KS], F8, name="q8")
        for lc in range(NLC):
            jsl = slice(lc * LCW, (lc + 1) * LCW)
            for d in range(2):
                nc.sync.dma_start(q8[:, d:d + 1, jsl],
                                  qsh8_in[d * 128:(d + 1) * 128, jsl])
            for d in range(2):
                nc.sync.dma_start(qb[:, d:d + 1, jsl],
                                  qsh_in[d * 128:(d + 1) * 128, jsl])

        # qlp tiles: DMAs are issued rate-matched inside the phase-1 loop
        # (two groups per simq chunk) so they neither starve phase-1's
        # write-backs nor leave phase 2 waiting
        qls = [qstg.tile([128, 4, C1], F8, name=f"ql{g}") for g in range(NG)]

        # ET (exp(simk/T_DC)) fp8, [128, NJT, 512]
        et = pool.tile([128, NJT, B], F8, name="et")

        # phase 1: per j-tile-pair simk (fp8 DoubleRow over d), exp, simq
        # (bf16). Pairing makes each exp/cast instruction 1024 wide, halving
        # per-instruction overhead on the gating Scalar/Vector engines.
        with ExitStack() as ph1:
            psk = ph1.enter_context(
                tc.tile_pool(name="psk", bufs=2, space="PSUM"))
            psq = ph1.enter_context(
                tc.tile_pool(name="psq", bufs=2, space="PSUM"))
            for ch in range(NCH):
                sqt = sq.tile([128, NJT // NCH, B], BF16, name="sqt", tag="sqt")
                for sp in range(NJT // NCH // 2):
                    t = ch * (NJT // NCH) + 2 * sp
                    pk = psk.tile([128, 2 * B], F32, name="pk", tag="pk")
                    pq = psq.tile([128, 2 * B], F32, name="pq", tag="pq")
                    for h in range(2):
                        jl = (t + h) * 128
                        nc.tensor.matmul(
                            pk[:, h * B:(h + 1) * B],
                            q8[:, :, jl:jl + 128], kT8[:, :, :],
                            start=True, stop=True, perf_mode=DR)
                    nc.scalar.activation(et[:, t:t + 2, :], pk[:],
                                         Act.Exp, scale=1.0 / T_DC)
                    for h in range(2):
                        jl = (t + h) * 128
                        for d in range(2):
                            nc.tensor.matmul(
                                pq[:, h * B:(h + 1) * B],
                                qb[:, d:d + 1, jl:jl + 128],
                                qTb[:, d:d + 1, :],
                                start=(d == 0), stop=(d == 1))
                    nc.vector.tensor_copy(
                        sqt[:, 2 * sp:2 * sp + 2, :], pq[:])
                nc.sync.dma_start(
                    simq_out[ch * 128:(ch + 1) * 128, :, :], sqt[:])
                for g in (2 * ch, 2 * ch + 1):
                    nc.sync.dma_start(
                        qls[g][:], qlp8_in[g * 128:(g + 1) * 128, :, :])

        # phase 2: P[b, cls+] = ET^T @ qlp8_aug, DoubleRow over 32 super-tiles
        with ExitStack() as ph2:
            ps2 = ph2.enter_context(
                tc.tile_pool(name="ps2", bufs=1, space="PSUM"))
            # [128, 1024] f32 = exactly 2 banks per tile: each matmul chunk
            # ([:, :512] / [:, 512:C1]) stays within one bank
            pacc = [ps2.tile([128, 1024], F32, name=f"pacc{bt}")
                    for bt in range(4)]
            # bt outer: each P accumulator finishes early so its copy-out
            # and DMA overlap the next bt's matmul stream
            for bt in range(4):
                for g in range(NG):
                    ql = qls[g]
                    for sl in range(2):
                        s = 2 * g + sl
                        st = (s == 0)
                        sp = (s == NST - 1)
                        lhs = et[:, 2 * s:2 * s + 2, bt * 128:(bt + 1) * 128]
                        nc.tensor.matmul(
                            pacc[bt][:, :512], lhs,
                            ql[:, 2 * sl:2 * sl + 2, :512],
                            start=st, stop=sp, perf_mode=DR)
                        nc.tensor.matmul(
                            pacc[bt][:, 512:C1], lhs,
                            ql[:, 2 * sl:2 * sl + 2, 512:],
                            start=st, stop=sp, perf_mode=DR)
                pcp = pc.tile([128, C1], F32, name="pcp", tag="pcp")
                if bt % 2 == 0:
                    nc.vector.tensor_copy(pcp[:], pacc[bt][:, :C1])
                else:
                    nc.scalar.activation(pcp[:], pacc[bt][:, :C1],
                                         Act.Copy, scale=1.0)
                nc.sync.dma_start(p_out[bt * 128:(bt + 1) * 128, :], pcp[:])

    _CACHED['nc'] = nc
    return nc


def _prep_inputs(norm_q, k_feat, queue, qlp):
    """Host-side cast + layout. Returns (in_maps, S)."""
    mx = float(qlp.max())
    S = float(2.0 ** np.floor(np.log2(200.0 / max(mx, 1e-20))))
    qT = np.ascontiguousarray(norm_q.T).astype(NP_BF16)
    kT8 = np.ascontiguousarray(k_feat.T).astype(NP_F8)
    in_maps = []
    for c in range(N_CORES):
        sh = slice(c * KS, (c + 1) * KS)
        qs = np.ascontiguousarray(queue[:, sh])
        # qlp shard -> [KS, C1] fp8 with ones col + pad, then grouped
        # DoubleRow interleave [NG*128, 4, C1]
        aug = np.zeros((KS, C1), np.float32)
        aug[:, :C] = qlp[:, sh].T * S
        aug[:, C] = 1.0
        aug8 = aug.astype(NP_F8)
        qlp8 = np.ascontiguousarray(
            aug8.reshape(NG, 2, 2, 128, C1).transpose(0, 3, 1, 2, 4)
        ).reshape(NG * 128, 4, C1)
        in_maps.append({
            "qT": qT, "kT8": kT8,
            "qsh": qs.astype(NP_BF16),
            "qsh8": qs.astype(NP_F8),
            "qlp8": qlp8,
        })
    return in_maps, S


def kernel(norm_q, q_logits, k_feat, logits_k, queue, queue_label_prob,
           queue_label, target, knn_k):
    norm_q = np.asarray(norm_q, np.float32)
    q_logits = np.asarray(q_logits, np.float32)
    k_feat = np.asarray(k_feat, np.float32)
    queue = np.asarray(queue, np.float32)
    qlp = np.asarray(queue_label_prob, np.float32)
    queue_label = np.asarray(queue_label)
    target = np.asarray(target)
    kk = int(knn_k)

    nc = _build()
    in_maps, S = _prep_inputs(norm_q, k_feat, queue, qlp)
    res = run_bass_kernel_spmd(nc, in_maps, list(range(N_CORES)))

    # simq[c, p, sub, b] -> sim rows j = c*1024 + sub*128 + p
    blocks = []
    for c in range(N_CORES):
        A = res.results[c]["simq"].astype(np.float32)
        A = A.reshape(NCH, 128, NJT // NCH, B).transpose(0, 2, 1, 3)
        blocks.append(A.reshape(KS, B).T)
    sim = np.concatenate(blocks, axis=1)
    P = np.zeros((B, C1), np.float64)
    for c in range(N_CORES):
        P += res.results[c]["pout"].astype(np.float64)

    # ---- supcon (exact top-k on the device-computed sim) ----
    idx = np.argpartition(-sim, kk - 1, axis=1)[:, :kk]
    sim_knn = np.take_along_axis(sim, idx, axis=1)
    w = np.exp((sim_knn - sim_knn.max(axis=1, keepdims=True)) / T_SUP)
    w /= w.sum(axis=1, keepdims=True)
    pos = (target[:, None] == queue_label[idx])
    gt = (w * pos).sum(axis=1)
    m = gt > EPS
    supin_loss = np.where(m, -np.log(np.where(m, gt, 1.0)), 0.0).sum() / B

    # ---- fc loss ----
    x = q_logits.astype(np.float64)
    lse = np.log(np.exp(x - x.max(1, keepdims=True)).sum(1)) + x.max(1)
    log_q = x - lse[:, None]
    q_mask = (x.min(1) - lse) > np.log(EPS)
    onehot = np.full((B, C), LS / (C - 1))
    onehot[np.arange(B), target] = 1.0 - LS
    fc_loss = -((onehot * log_q).sum(1) * q_mask).sum() / B

    # ---- dc loss ----
    Z = P[:, C] * S
    dc_t = P[:, :C] / Z[:, None]
    dc_pos = dc_t > 0
    kl = np.where(dc_pos,
                  dc_t * (np.log(np.where(dc_pos, dc_t, 1.0)) - log_q), 0.0)
    dc_loss = (kl.sum(1) * q_mask).sum() / B

    return (np.float32(supin_loss), np.float32(fc_loss), np.float32(dc_loss))


# revision 18
# speedup vs baseline: 1.4864x; 1.1063x over previous
"""Trainium2 Bass kernel for nn_CoNe_35974646071945 (retrieval_knn).

Strategy: K-shard the 65536-entry queue across 8 NeuronCores. Host pre-casts
all inputs to fp8 e4m3 so the device reads a quarter of the f32 bytes and
every matmul runs in fp8 DoubleRow mode. Each core:
  simqT[j, b] = (queue8_shard)^T @ norm_q8^T    (fp8 DR matmul -> bf16 out)
  simkT[j, b] = (queue8_shard)^T @ k_feat8^T    (fp8 DR matmul)
  ET[j, b]    = exp(simkT / T_DC)               (fp8, SBUF-resident)
  P[b, cls+]  = ET^T @ [qlp8_shard^T * S | 1]   (fp8 DR matmuls, fp32 PSUM
                                                 accumulated over the shard;
                                                 ones column gives the
                                                 softmax partition Z)
Host: the device sims only PRESELECT top-M=2*knn candidates per row; the
host recomputes exact f32 sims for those M and does the exact top-200 /
softmax on them (validated: zero membership misses, supin exact to 1e-7).
P partials are summed over cores; softmax/KL on tiny arrays.

Layout notes: queue loads are chunked along j so the first matmuls start
~2us in; simq writes are batched 8 j-tiles per DMA; qlp streams in 16
half-MB groups, triggers rate-matched (last 4 phase-1 chunks + 6-group
lead inside phase 2's first bt pass) so they never starve phase-1
write-backs nor phase 2. All PSUM matmul outputs are single-bank (the
512/496 split of the 1008-wide dc accumulation is load-bearing: a chunk
straddling a 2KB PSUM bank boundary corrupts the accumulation).
"""
import sys
sys.path.insert(0, '/opt/trn_rl_repo')
sys.path.insert(0, '/root/.axon_site/_ro/trn_rl_repo')

import numpy as np
import ml_dtypes
from contextlib import ExitStack

from concourse import bass, tile, mybir
from concourse.bass_utils import run_bass_kernel_spmd
from concourse.vector_clock import ScopedClock, VectorClock

F32 = mybir.dt.float32
BF16 = mybir.dt.bfloat16
F8 = mybir.dt.float8e4
Alu = mybir.AluOpType
Act = mybir.ActivationFunctionType
DR = mybir.MatmulPerfMode.DoubleRow

NP_BF16 = ml_dtypes.bfloat16
NP_F8 = ml_dtypes.float8_e4m3

N_CORES = 8
B, D, K, C = 512, 256, 65536, 1000
KS = K // N_CORES            # 8192 queue columns per core
T_SUP, T_DC, LS = 0.07, 0.1, 0.1
EPS = 1e-8
NJT = KS // 128              # 64 j-tiles per core
NST = NJT // 2               # 32 super-tiles (256 j) for DoubleRow
NG = NST // 2                # 16 qlp DMA groups (512 j each)
C1 = 1008                    # 1000 classes + ones col (idx 1000) + pad
NCH = 8                      # simq output chunks (8 j-tiles each)
NLC = 4                      # queue load chunks (2048 j-cols each)
LCW = KS // NLC


class CompatTileContext(tile.TileContext):
    """This walrus build encodes at most ONE sync wait per instruction.
    Split Tile's multi-wait instructions and its tail drain."""

    def _commit_instruction(self, inst, lazy_reg_writes=True):
        si = inst.sync_info
        if (
            si is not None
            and si.on_wait
            and len(si.on_wait) > 1
            and inst.engine != mybir.EngineType.Unassigned
        ):
            import bass_rust
            waits = list(si.on_wait)
            for w in waits[:-1]:
                nop = mybir.InstNoOp(
                    name=f"I-{self.nc.next_id()}", ins=[], outs=[]
                )
                nop.engine = inst.engine
                nop.sync_info = bass_rust.SyncInfo(on_wait=[w], on_update=[])
                super()._commit_instruction(nop, lazy_reg_writes=False)
            si.on_wait = [waits[-1]]
            inst.sync_info = si
        super()._commit_instruction(inst, lazy_reg_writes=lazy_reg_writes)

    def _drain_and_barrier(self, tick_clock, wait_clock):
        gclock = tick_clock.global_clock
        n = len(gclock)
        for i in range(n):
            if gclock[i] == 0:
                continue
            vec = [0] * n
            vec[i] = gclock[i]
            nop_inst = self.nc.sync.nop(nofuse=True, hint=f"tail_wait_p{i}")
            wait_clock.add_sem_waits(
                nop_inst.ins, ScopedClock({None: VectorClock(vec)})
            )
        self.nc.sync.drain()
        self.nc.all_engine_barrier()
        assert self.sems is not None
        popped = self.nc._tile_sem_poison_stack.pop()
        assert popped is self._sem_poison
        self.nc.clear_and_free_semaphores(list(self.sems.allocated().values()))
        self.nc.all_engine_barrier()


_CACHED = {}


def _build():
    if 'nc' in _CACHED:
        return _CACHED['nc']
    nc = bass.Bass(num_devices=N_CORES)
    qT8_in = nc.declare_dram_parameter("qT8", [D, B], F8, isOutput=False)
    kT8_in = nc.declare_dram_parameter("kT8", [D, B], F8, isOutput=False)
    qsh8_in = nc.declare_dram_parameter("qsh8", [D, KS], F8, isOutput=False)
    qlp8_in = nc.declare_dram_parameter(
        "qlp8", [NG * 128, 4, C1], F8, isOutput=False)
    # simq[c*128+p, sub, b] = sim[j = c*1024 + sub*128 + p, b]
    simq_out = nc.declare_dram_parameter(
        "simq", [NCH * 128, NJT // NCH, B], BF16, isOutput=True)
    p_out = nc.declare_dram_parameter("pout", [B, C1], F32, isOutput=True)

    with ExitStack() as ctx:
        tc = ctx.enter_context(CompatTileContext(nc))
        pool = ctx.enter_context(tc.tile_pool(name="main", bufs=1))
        qstg = ctx.enter_context(tc.tile_pool(name="qstg", bufs=1))
        sq = ctx.enter_context(tc.tile_pool(name="sq", bufs=3))
        pc = ctx.enter_context(tc.tile_pool(name="pc", bufs=4))

        # moving operands first: norm_q^T / k_feat^T fp8, [d, 2, b]
        qT8 = pool.tile([128, 2, B], F8, name="qT8")
        kT8 = pool.tile([128, 2, B], F8, name="kT8")
        for d in range(2):
            nc.sync.dma_start(kT8[:, d:d + 1, :], kT8_in[d * 128:(d + 1) * 128, :])
            nc.sync.dma_start(qT8[:, d:d + 1, :], qT8_in[d * 128:(d + 1) * 128, :])

        # stationary queue shard, chunked along j so matmuls start early
        q8 = pool.tile([128, 2, KS], F8, name="q8")
        for lc in range(NLC):
            jsl = slice(lc * LCW, (lc + 1) * LCW)
            for d in range(2):
                nc.sync.dma_start(q8[:, d:d + 1, jsl],
                                  qsh8_in[d * 128:(d + 1) * 128, jsl])

        # qlp tiles: DMAs are issued rate-matched inside the phase-1 loop
        # (two groups per simq chunk) so they neither starve phase-1's
        # write-backs nor leave phase 2 waiting
        qls = [qstg.tile([128, 4, C1], F8, name=f"ql{g}") for g in range(NG)]

        # ET (exp(simk/T_DC)) fp8, [128, NJT, 512]
        et = pool.tile([128, NJT, B], F8, name="et")

        # phase 1: per j-tile-pair simk (fp8 DoubleRow over d), exp, simq
        # (bf16). Pairing makes each exp/cast instruction 1024 wide, halving
        # per-instruction overhead on the gating Scalar/Vector engines.
        with ExitStack() as ph1:
            psk = ph1.enter_context(
                tc.tile_pool(name="psk", bufs=2, space="PSUM"))
            psq = ph1.enter_context(
                tc.tile_pool(name="psq", bufs=2, space="PSUM"))
            for ch in range(NCH):
                sqt = sq.tile([128, NJT // NCH, B], BF16, name="sqt", tag="sqt")
                for sp in range(NJT // NCH // 2):
                    t = ch * (NJT // NCH) + 2 * sp
                    pk = psk.tile([128, 2 * B], F32, name="pk", tag="pk")
                    pq = psq.tile([128, 2 * B], F32, name="pq", tag="pq")
                    for h in range(2):
                        jl = (t + h) * 128
                        nc.tensor.matmul(
                            pk[:, h * B:(h + 1) * B],
                            q8[:, :, jl:jl + 128], kT8[:, :, :],
                            start=True, stop=True, perf_mode=DR)
                        nc.tensor.matmul(
                            pq[:, h * B:(h + 1) * B],
                            q8[:, :, jl:jl + 128], qT8[:, :, :],
                            start=True, stop=True, perf_mode=DR)
                    nc.scalar.activation(et[:, t:t + 2, :], pk[:],
                                         Act.Exp, scale=1.0 / T_DC)
                    nc.vector.tensor_copy(
                        sqt[:, 2 * sp:2 * sp + 2, :], pq[:])
                nc.sync.dma_start(
                    simq_out[ch * 128:(ch + 1) * 128, :, :], sqt[:])
                if ch >= NCH - 4:
                    for g in (2 * (ch - NCH + 4), 2 * (ch - NCH + 4) + 1):
                        nc.sync.dma_start(
                            qls[g][:], qlp8_in[g * 128:(g + 1) * 128, :, :])

        # phase 2: P[b, cls+] = ET^T @ qlp8_aug, DoubleRow over 32 super-tiles
        with ExitStack() as ph2:
            ps2 = ph2.enter_context(
                tc.tile_pool(name="ps2", bufs=1, space="PSUM"))
            # [128, 1024] f32 = exactly 2 banks per tile: each matmul chunk
            # ([:, :512] / [:, 512:C1]) stays within one bank
            pacc = [ps2.tile([128, 1024], F32, name=f"pacc{bt}")
                    for bt in range(4)]
            # bt outer: each P accumulator finishes early so its copy-out
            # and DMA overlap the next bt's matmul stream
            for bt in range(4):
                for g in range(NG):
                    if bt == 0 and 8 <= g + 6 < NG:
                        gg = g + 6
                        nc.sync.dma_start(
                            qls[gg][:],
                            qlp8_in[gg * 128:(gg + 1) * 128, :, :])
                    ql = qls[g]
                    for sl in range(2):
                        s = 2 * g + sl
                        st = (s == 0)
                        sp = (s == NST - 1)
                        lhs = et[:, 2 * s:2 * s + 2, bt * 128:(bt + 1) * 128]
                        nc.tensor.matmul(
                            pacc[bt][:, :512], lhs,
                            ql[:, 2 * sl:2 * sl + 2, :512],
                            start=st, stop=sp, perf_mode=DR)
                        nc.tensor.matmul(
                            pacc[bt][:, 512:C1], lhs,
                            ql[:, 2 * sl:2 * sl + 2, 512:],
                            start=st, stop=sp, perf_mode=DR)
                pcp = pc.tile([128, C1], F32, name="pcp", tag="pcp")
                if bt % 2 == 0:
                    nc.vector.tensor_copy(pcp[:], pacc[bt][:, :C1])
                else:
                    nc.scalar.activation(pcp[:], pacc[bt][:, :C1],
                                         Act.Copy, scale=1.0)
                nc.sync.dma_start(p_out[bt * 128:(bt + 1) * 128, :], pcp[:])

    _CACHED['nc'] = nc
    return nc


def _prep_inputs(norm_q, k_feat, queue, qlp):
    """Host-side cast + layout. Returns (in_maps, S)."""
    mx = float(qlp.max())
    S = float(2.0 ** np.floor(np.log2(200.0 / max(mx, 1e-20))))
    qT8 = np.ascontiguousarray(norm_q.T).astype(NP_F8)
    kT8 = np.ascontiguousarray(k_feat.T).astype(NP_F8)
    in_maps = []
    for c in range(N_CORES):
        sh = slice(c * KS, (c + 1) * KS)
        # qlp shard -> [KS, C1] fp8 with ones col + pad, then grouped
        # DoubleRow interleave [NG*128, 4, C1]
        aug = np.zeros((KS, C1), np.float32)
        aug[:, :C] = qlp[:, sh].T * S
        aug[:, C] = 1.0
        aug8 = aug.astype(NP_F8)
        qlp8 = np.ascontiguousarray(
            aug8.reshape(NG, 2, 2, 128, C1).transpose(0, 3, 1, 2, 4)
        ).reshape(NG * 128, 4, C1)
        in_maps.append({
            "qT8": qT8, "kT8": kT8,
            "qsh8": np.ascontiguousarray(queue[:, sh]).astype(NP_F8),
            "qlp8": qlp8,
        })
    return in_maps, S


def kernel(norm_q, q_logits, k_feat, logits_k, queue, queue_label_prob,
           queue_label, target, knn_k):
    norm_q = np.asarray(norm_q, np.float32)
    q_logits = np.asarray(q_logits, np.float32)
    k_feat = np.asarray(k_feat, np.float32)
    queue = np.asarray(queue, np.float32)
    qlp = np.asarray(queue_label_prob, np.float32)
    queue_label = np.asarray(queue_label)
    target = np.asarray(target)
    kk = int(knn_k)

    nc = _build()
    in_maps, S = _prep_inputs(norm_q, k_feat, queue, qlp)
    res = run_bass_kernel_spmd(nc, in_maps, list(range(N_CORES)))

    # simq[c, p, sub, b] -> sim rows j = c*1024 + sub*128 + p
    blocks = []
    for c in range(N_CORES):
        A = res.results[c]["simq"].astype(np.float32)
        A = A.reshape(NCH, 128, NJT // NCH, B).transpose(0, 2, 1, 3)
        blocks.append(A.reshape(KS, B).T)
    sim = np.concatenate(blocks, axis=1)
    P = np.zeros((B, C1), np.float64)
    for c in range(N_CORES):
        P += res.results[c]["pout"].astype(np.float64)

    # ---- supcon: coarse top-M from device fp8 sims, exact f32 refine ----
    M = min(max(2 * kk, kk + 128), K)
    cidx = np.argpartition(-sim, M - 1, axis=1)[:, :M]
    gath = queue.T[cidx]                       # [B, M, D]
    ref = np.einsum('bmd,bd->bm', gath, norm_q)
    sel = np.argpartition(-ref, kk - 1, axis=1)[:, :kk]
    idx = np.take_along_axis(cidx, sel, axis=1)
    sim_knn = np.take_along_axis(ref, sel, axis=1)
    w = np.exp((sim_knn - sim_knn.max(axis=1, keepdims=True)) / T_SUP)
    w /= w.sum(axis=1, keepdims=True)
    pos = (target[:, None] == queue_label[idx])
    gt = (w * pos).sum(axis=1)
    m = gt > EPS
    supin_loss = np.where(m, -np.log(np.where(m, gt, 1.0)), 0.0).sum() / B

    # ---- fc loss ----
    x = q_logits.astype(np.float64)
    lse = np.log(np.exp(x - x.max(1, keepdims=True)).sum(1)) + x.max(1)
    log_q = x - lse[:, None]
    q_mask = (x.min(1) - lse) > np.log(EPS)
    onehot = np.full((B, C), LS / (C - 1))
    onehot[np.arange(B), target] = 1.0 - LS
    fc_loss = -((onehot * log_q).sum(1) * q_mask).sum() / B

    # ---- dc loss ----
    Z = P[:, C] * S
    dc_t = P[:, :C] / Z[:, None]
    dc_pos = dc_t > 0
    kl = np.where(dc_pos,
                  dc_t * (np.log(np.where(dc_pos, dc_t, 1.0)) - log_q), 0.0)
    dc_loss = (kl.sum(1) * q_mask).sum() / B

    return (np.float32(supin_loss), np.float32(fc_loss), np.float32(dc_loss))
